# revision 30
# baseline (speedup 1.0000x reference)
"""Binary-weight 3x3 SAME conv (NHWC) on Trainium2, data-parallel over 8 cores.

Problem: x (32,56,56,256) f32, w (3,3,256,256) f32.
  out = conv2d(x, sign(clip(w,-1,1)), SAME, stride 1)   # NHWC / HWIO

Strategy (per core, 4 images):
  - Binarize w on device (DVE: 2*(w>=0)-1 -> bf16, +-1 exact).
  - DMA x tiles [112pos, 256ci] f32, cast to bf16 (DVE), TensorE-transpose to
    channel-major xt [128ci, 2, 4*58*58] bf16; each image is a zero-padded
    58x58 plane (only the pad strips are memset), so SAME padding becomes
    plain reads. bf16 rounding of x is the only precision loss.
  - Conv = 9 shifted matmuls accumulated in PSUM per output chunk of 8 rows:
      psum[128co, 448] += s[tap][ci,co].T @ xt[ci, shifted 8x56 window]
    Transposes are interleaved at chunk granularity so the PE starts conv
    work as soon as the first weight+activation tiles land (~11us, DMA-bound).
  - Output written channel-major (2,128co,4b,3136pos) f32; host transposes.

Built with bacc.Bacc + nc.compile(): walrus allows only one sync wait per
instruction, and Bacc's move_matmul_waits_to_ldweights/generate_event_semaphores
passes enforce that.
"""

import numpy as np

import concourse.bacc as bacc
import concourse.mybir as mybir
import concourse.tile as tile

# ---- problem constants (hardcoded; kernel.py must be self-contained) ----
B_FULL, H, W, CI, CO, K = 32, 56, 56, 256, 256, 3
N_CORES = 8
B = B_FULL // N_CORES          # 4 images per core
IMG = H * W                    # 3136 valid positions per image
P = 128
HP, WP = H + 2, W + 2          # 58x58 zero-padded plane per image
IMGP = HP * WP                 # 3364
POSP = B * IMGP                # 13456 padded positions per core
TROWS = 2                      # image rows per transpose tile
TPOS = TROWS * W               # 112 positions per transpose tile
NT_IMG = H // TROWS            # 28 transpose tiles per image
CI_C = CI // P                 # 2 contraction chunks
CO_C = CO // P                 # 2 output-channel chunks
YCHUNK = 8                     # output rows per psum tile
NCHUNK = H // YCHUNK           # 7 chunks per image
FREE = YCHUNK * W              # 448 <= 512 psum fp32 bank limit

F32 = mybir.dt.float32
BF16 = mybir.dt.bfloat16


def _emit_body(nc, pools, x_d, w_d, o_d):
    import ml_dtypes

    (const_pool, ws_pool, win_pool, xin_pool, xc_pool, xt_pool, out_pool,
     tpsum_pool, cpsum_pool) = pools

    x_flat = x_d.ap().flatten_outer_dims()      # [B*IMG, CI]

    # identity via inline const (keeps gpsimd out of the program); bf16 so
    # transposes run at 1 cycle/row on the PE.
    ident_dram = nc.inline_tensor(np.eye(P, dtype=ml_dtypes.bfloat16), name="ident_c")
    ident = const_pool.tile([P, P], BF16, name="ident")
    nc.sync.dma_start(out=ident, in_=ident_dram.ap())

    # ---- binarize weights: s_all [128ci, 9tap*2cc, 256co], split 4 ways so
    # the DMA spreads across queues and the DVE sign pipeline starts early ----
    w_src = w_d.ap().rearrange("ky kx (cc p) co -> p (ky kx cc) co", p=P)
    wtile = win_pool.tile([P, K * K * CI_C, CO], F32, name="wtile")
    s_all = ws_pool.tile([P, K * K * CI_C, CO], BF16, name="s_all")
    w_bounds = [0, 5, 10, 14, 18]
    for a, bnd in zip(w_bounds[:-1], w_bounds[1:]):
        nc.sync.dma_start(out=wtile[:, a:bnd], in_=w_src[:, a:bnd])

    def emit_signs():
        for a, bnd in zip(w_bounds[:-1], w_bounds[1:]):
            # sign(w) = 2*(w >= 0) - 1 (exact +-1 in bf16); on DVE so conv
            # matmuls only wait on the DVE semaphore.
            nc.vector.tensor_scalar(
                s_all[:, a:bnd], wtile[:, a:bnd], 0.0, None, mybir.AluOpType.is_ge
            )
            nc.vector.tensor_scalar(
                s_all[:, a:bnd], s_all[:, a:bnd], 2.0, -1.0,
                mybir.AluOpType.mult, mybir.AluOpType.add,
            )

    def s_tile(t, cc, oc):
        return s_all[:, t * CI_C + cc, oc * P : (oc + 1) * P]

    # ---- channel-major activations, bf16, zero-padded 58x58 planes ----
    xt = xt_pool.tile([P, CI_C, POSP], BF16, name="xt")
    xt_plane = xt.rearrange("p c (b y x) -> p c b y x", y=HP, x=WP)

    # zero only the pad strips (top/bottom rows, left/right cols); gpsimd is
    # otherwise idle so this costs nothing on the critical path
    for b in range(B):
        for cc in range(CI_C):
            nc.gpsimd.memset(xt_plane[:, cc, b, 0, :], 0.0)
            nc.gpsimd.memset(xt_plane[:, cc, b, HP - 1, :], 0.0)
            nc.gpsimd.memset(xt_plane[:, cc, b, 1 : HP - 1, 0], 0.0)
            nc.gpsimd.memset(xt_plane[:, cc, b, 1 : HP - 1, WP - 1], 0.0)

    # HWDGE descriptor generation costs ~625ns PER dma_start, serialized on
    # one dispatcher — so batch 4 transpose tiles (8 image rows) per input DMA
    # and 2 output chunks per output DMA to keep the instruction count low.
    TBLK = 4                       # transpose tiles per input DMA
    NBLK_IMG = NT_IMG // TBLK      # 7 blocks per image
    N_BLKS = B * NBLK_IMG
    emitted = [0]

    def emit_transposes(upto_blk):
        for g in range(emitted[0], min(N_BLKS, upto_blk)):
            b, blk = divmod(g, NBLK_IMG)
            xin = xin_pool.tile([TPOS, TBLK, CI], F32, name="xin", tag="xin")
            src0 = b * IMG + blk * TBLK * TPOS
            nc.sync.dma_start(
                out=xin,
                in_=x_flat[src0 : src0 + TBLK * TPOS, :].rearrange(
                    "(k p) c -> p k c", k=TBLK
                ),
            )
            xc = xc_pool.tile([TPOS, TBLK, CI], BF16, name="xc", tag="xc")
            nc.vector.tensor_copy(out=xc, in_=xin)
            for k in range(TBLK):
                r0 = (blk * TBLK + k) * TROWS + 1  # padded row of first element
                for cc in range(CI_C):
                    tps = tpsum_pool.tile([P, TPOS], BF16, name="tps", tag="tps")
                    nc.tensor.transpose(
                        tps, xc[:, k, cc * P : (cc + 1) * P], ident[:TPOS, :TPOS]
                    )
                    nc.vector.tensor_copy(
                        out=xt_plane[:, cc, b, r0 : r0 + TROWS, 1 : 1 + W],
                        in_=tps.rearrange("p (r x) -> p r x", x=W),
                    )
        emitted[0] = max(emitted[0], min(N_BLKS, upto_blk))

    LOOKAHEAD = 2  # transpose blocks emitted ahead of the consuming chunk

    # First casts/transposes go AHEAD of the sign ops in the DVE queue: the
    # signs block on the 2.3MB weight DMA, and queueing them first would stall
    # the transpose pipeline (and the PE) for the whole weight-load latency.
    emit_transposes(2)
    emit_signs()

    for b in range(B):
        xviews = [
            xt[:, cc, b * IMGP : (b + 1) * IMGP].rearrange("p (y x) -> p y x", x=WP)
            for cc in range(CI_C)
        ]
        ot2 = None
        for c in range(NCHUNK):
            y0 = c * YCHUNK
            # conv chunk c reads padded rows [y0, y0+10) = valid rows
            # [y0-1, y0+8] -> needs blocks covering image rows < y0+9
            need = b * NBLK_IMG + min(NBLK_IMG, -(-(y0 + YCHUNK + 1) // (TBLK * TROWS)))
            emit_transposes(need + LOOKAHEAD)
            for oc in range(CO_C):
                cps = cpsum_pool.tile([P, FREE], F32, name="cps", tag="cps")
                first = True
                for t in range(K * K):
                    ky, kx = divmod(t, K)
                    for cc in range(CI_C):
                        rhs = xviews[cc][:, y0 + ky : y0 + ky + YCHUNK, kx : kx + W]
                        nc.tensor.matmul(
                            cps,
                            s_tile(t, cc, oc),
                            rhs,
                            start=first,
                            stop=(t == K * K - 1 and cc == CI_C - 1),
                        )
                        first = False
                # psum->sbuf copy on the otherwise-idle ScalarE, keeping DVE
                # free for the transpose pipeline; pairs of chunks share one
                # output tile so the store is a single (cheaper) DMA.
                if c % 2 == 0:
                    if c == NCHUNK - 1:  # odd tail chunk
                        ot = out_pool.tile([P, FREE], F32, name="ot", tag="ot2")
                        nc.scalar.activation(
                            ot, cps, mybir.ActivationFunctionType.Copy
                        )
                        nc.sync.dma_start(
                            out=o_d.ap()[oc, :, b, y0 * W : (y0 + YCHUNK) * W],
                            in_=ot,
                        )
                    else:
                        ot2 = out_pool.tile(
                            [P, 2, FREE], F32, name=f"ot2_{oc}", tag="ot2"
                        )
                        nc.scalar.activation(
                            ot2[:, 0], cps, mybir.ActivationFunctionType.Copy
                        )
                        if oc == 0:
                            ot2_pair = [ot2, None]
                        else:
                            ot2_pair[1] = ot2
                else:
                    prev = ot2_pair[oc]
                    nc.scalar.activation(
                        prev[:, 1], cps, mybir.ActivationFunctionType.Copy
                    )
                    nc.sync.dma_start(
                        out=o_d.ap()[oc, :, b, (y0 - YCHUNK) * W : (y0 + YCHUNK) * W],
                        in_=prev.rearrange("p a f -> p (a f)"),
                    )


def build_program(reps: int = 1):
    # Bacc (not plain Bass): compile() runs move_matmul_waits_to_ldweights +
    # generate_event_semaphores, required because walrus allows only one sync
    # wait per instruction.
    nc = bacc.Bacc("TRN2", debug=False, num_devices=N_CORES)
    x_d = nc.dram_tensor("x", [B, H, W, CI], F32, kind="ExternalInput")
    w_d = nc.dram_tensor("w", [K, K, CI, CO], F32, kind="ExternalInput")
    o_d = nc.dram_tensor("out", [CO_C, P, B, IMG], F32, kind="ExternalOutput")

    with tile.TileContext(nc) as tc:
        with (
            tc.tile_pool(name="const", bufs=1) as const_pool,
            tc.tile_pool(name="ws", bufs=1) as ws_pool,
            tc.tile_pool(name="win", bufs=1) as win_pool,
            tc.tile_pool(name="xin", bufs=4) as xin_pool,
            tc.tile_pool(name="xcp", bufs=4) as xc_pool,
            tc.tile_pool(name="xtp", bufs=1) as xt_pool,
            tc.tile_pool(name="outs", bufs=4) as out_pool,
            tc.tile_pool(name="tpsum", bufs=3, space="PSUM") as tpsum_pool,
            tc.tile_pool(name="cpsum", bufs=5, space="PSUM") as cpsum_pool,
        ):
            pools = (const_pool, ws_pool, win_pool, xin_pool, xc_pool,
                     xt_pool, out_pool, tpsum_pool, cpsum_pool)
            if reps == 1:
                _emit_body(nc, pools, x_d, w_d, o_d)
            else:
                with tc.For_i(0, reps, 1):
                    _emit_body(nc, pools, x_d, w_d, o_d)
    nc.compile()
    return nc


_NC_CACHE = {}


def _get_program(reps: int = 1):
    if reps not in _NC_CACHE:
        _NC_CACHE[reps] = build_program(reps)
    return _NC_CACHE[reps]


def kernel(x: np.ndarray, w: np.ndarray) -> np.ndarray:
    from concourse.bass_utils import run_bass_kernel_spmd

    x = np.ascontiguousarray(x, dtype=np.float32)
    w = np.ascontiguousarray(w, dtype=np.float32)
    nc = _get_program()
    in_maps = [
        {"x": np.ascontiguousarray(x[c * B : (c + 1) * B]), "w": w}
        for c in range(N_CORES)
    ]
    res = run_bass_kernel_spmd(nc, in_maps, core_ids=list(range(N_CORES))).results
    outs = []
    for c in range(N_CORES):
        r = res[c]["out"]  # (CO_C, P, B, IMG)
        o = r.transpose(2, 3, 0, 1).reshape(B, H, W, CO)
        outs.append(o)
    return np.ascontiguousarray(np.concatenate(outs, axis=0))
